# revision 1
# baseline (speedup 1.0000x reference)
"""CrossAttention (B=4, N=M=2048, C=1024, H=16, D=64) on 8 TRN2 cores.

Sharding: core = 2*b + g  (b = batch 0..3, g = head-half 0..1, 8 heads each).
Each core computes attention for its 8 heads and a partial (full-width)
output projection over its 512 local channels; the host sums the two
partials per batch and transposes back.

Device layout notes:
  - All activations live transposed (channels on partitions) so every
    matmul has its contraction on the partition axis with no on-chip
    transposes.  The host feeds query/key/value pre-transposed and the
    weights pre-tiled to the exact SBUF layout (contiguous DMAs).
  - scoresT = K_h^T-stationary x qT-moving -> (m on partitions, n free);
    the two heads of a pair run as concurrent row-tiled matmuls
    (partitions 0-63 / 64-127 of the same d-tile).
  - softmax denominator comes free from a ones-column appended to V
    (M=65 AV matmul, row 64 of the accumulator is sum_m exp(s)).
  - exp on the scalar engine (psum->sbuf, width 1024).
  - AV runs in fp8e4m3 with DoubleRow (two key-tiles per matmul at
    double rate); V and exp(s) quantization noise largely cancels in
    the softmax ratio.
  - Normalization: DVE reciprocal of the denominator row + GPSIMD
    partition-broadcast + DVE multiply into xT.
  - Projection order q, k, v lets exp start while v still projects.
"""

from contextlib import ExitStack

import ml_dtypes
import numpy as np

import concourse.bass as bass
import concourse.mybir as mybir
import concourse.tile as tile
from concourse import bacc, library_config
from concourse.bass_utils import run_bass_kernel_spmd

dt = mybir.dt
AF = mybir.ActivationFunctionType

# Problem dims (hardcoded; must match the harness inputs).
B, N, M, C, H = 4, 2048, 2048, 1024, 16
D = C // H            # 64
SCALE = D ** -0.5     # 0.125 (exact)
CL = C // 2           # 512 channels per core (8 heads)
HL = H // 2           # 8 local heads
P = 128
CT = C // P           # 8 input-channel tiles
DT = CL // P          # 4 local-channel tiles
MT = M // P           # 16 key tiles
NCH = 512             # psum bank width in fp32
NCHUNKS = N // NCH    # 4
EXPW = 1024           # exp width (2 psum banks)
VA = D + 1            # 65: v columns + ones column

F32 = dt.float32
F32R = dt.float32r
BF16 = dt.bfloat16
FP8 = dt.float8e4


def build_program(reps: int = 1, mode: str = "") -> bass.Bass:
    """reps>1 repeats the whole body for timing (wall-time delta isolates
    device time from host/transfer overhead).

    mode flags (diagnostics): 'P' stop after projections, 'A' stop after
    attention, 'O' skip attention (zero xT), 'X' skip final output DMA,
    'V' bf16 M=65 AV instead of fp8 DoubleRow, 'U' unpacked scores."""
    nc = bacc.Bacc()
    nc.gpsimd.load_library(library_config.attn)

    fp8_av = "8" in mode      # fp8 DoubleRow AV costs ~4% rel err; off
    vdt = FP8 if fp8_av else BF16

    qTin = nc.declare_dram_parameter("qTin", [C, N], F32R, isOutput=False)
    kTin = nc.declare_dram_parameter("kTin", [C, M], F32R, isOutput=False)
    vTin = nc.declare_dram_parameter("vTin", [C, M], F32R, isOutput=False)
    # weights arrive pre-tiled to SBUF layout: contiguous (P, x) DMAs
    wq = nc.declare_dram_parameter("wq", [P, CT * CL], F32R, isOutput=False)
    wk = nc.declare_dram_parameter("wk", [P, CT * CL], F32R, isOutput=False)
    wv = nc.declare_dram_parameter("wv", [P, CT * CL], F32R, isOutput=False)
    wp = nc.declare_dram_parameter("wp", [P, DT * C], BF16, isOutput=False)
    bp = nc.declare_dram_parameter("bp", [P, CT], F32, isOutput=False)
    out = nc.declare_dram_parameter("out", [C, N], F32, isOutput=True)

    with tile.TileContext(nc) as tc:
      for _rep in range(reps):
       with ExitStack() as ctx:
        # ---- persistent sbuf tensors -------------------------------------
        const_pool = ctx.enter_context(tc.tile_pool(name="consts", bufs=1))
        bp_sb = const_pool.tile([P, CT], F32)
        nbias = const_pool.tile([P, 1], F32)
        nc.vector.memset(nbias[:], -4.0 if fp8_av else -2.0)
        qT_sb = const_pool.tile([P, DT * N], F32R)   # local q, transposed
        kT_sb = const_pool.tile([P, DT * M], F32R)   # local k, transposed
        # v(+ones) per m-tile-pair, head, plane: [P][pair][h][plane][VA]
        va_sb = const_pool.tile([P, (MT // 2) * HL * 2 * VA], vdt)
        xT_sb = const_pool.tile([P, DT * N], BF16)  # attention out, transposed

        nc.sync.dma_start(out=bp_sb[:], in_=bp[:, :])

        va4 = va_sb[:].rearrange("p (r h t e) -> p r h t e", h=HL, t=2, e=VA)

        # ---- attention pools; pt opens before phase 1 so exp can run
        #      while v still projects (scores only need qT/kT + psum) ----
        ps_pool = ctx.enter_context(tc.tile_pool(name="ps2", bufs=2, space="PSUM"))
        av_pool = ctx.enter_context(tc.tile_pool(name="av", bufs=2, space="PSUM"))
        pt_pool = ctx.enter_context(
            tc.tile_pool(name="pt", bufs=10 if fp8_av else 7))

        # ---- phase 1: projections (q, k halved; v with all tiles) ------
        with tc.tile_pool(name="inT", bufs=8) as in_pool, \
             tc.tile_pool(name="wcur", bufs=1) as w_pool:

            def load_ctile(src, ct):
                t = in_pool.tile([P, N], F32R, tag="inT", name=f"i{ct}")
                nc.sync.dma_start(out=t[:], in_=src[ct * P:(ct + 1) * P, :])
                return t

            # --- q and k projections (halved accumulation) ---
            for src, w_dram, dst_sb in ((qTin, wq, qT_sb), (kTin, wk, kT_sb)):
                w_sb = w_pool.tile([P, CT * CL], F32R, tag="w", name="w")
                nc.sync.dma_start(out=w_sb[:], in_=w_dram[:, :])
                for half in range(2):
                    tiles = [load_ctile(src, half * 4 + ci) for ci in range(4)]
                    for j in range(DT):
                        for ch in range(NCHUNKS):
                            acc = ps_pool.tile([P, EXPW], F32, tag="big",
                                               name="prj")
                            for ci in range(4):
                                ct = half * 4 + ci
                                nc.tensor.matmul(
                                    acc[:, :NCH],
                                    w_sb[:, ct * CL + j * P: ct * CL + (j + 1) * P],
                                    tiles[ci][:, ch * NCH:(ch + 1) * NCH],
                                    start=(ci == 0),
                                    stop=(ci == 3),
                                )
                            dst = dst_sb[:, j * N + ch * NCH:
                                         j * N + (ch + 1) * NCH]
                            if half == 0:
                                nc.vector.tensor_copy(dst, acc[:, :NCH])
                            else:
                                nc.vector.tensor_add(dst, acc[:, :NCH], dst)

            # --- v projection (natural orientation, all c-tiles live) ---
            wv_sb = w_pool.tile([P, CT * CL], F32R, tag="w", name="w")
            nc.sync.dma_start(out=wv_sb[:], in_=wv[:, :])
            vtiles = [load_ctile(vTin, ct) for ct in range(CT)]
            for mt in range(MT):
                acc = ps_pool.tile([P, EXPW], F32, tag="big", name="prv")
                for ct in range(CT):
                    nc.tensor.matmul(
                        acc[:, :CL],
                        vtiles[ct][:, mt * P:(mt + 1) * P],
                        wv_sb[:, ct * CL:(ct + 1) * CL],
                        start=(ct == 0),
                        stop=(ct == CT - 1),
                    )
                blk = va4[:, mt // 2, :, mt % 2, :]       # (P, HL, VA)
                nc.vector.tensor_copy(
                    blk[:, :, :D],
                    acc[:, :CL].rearrange("p (h d) -> p h d", d=D),
                )
                nc.vector.memset(blk[:, :, D:VA], 1.0)

        if "P" in mode:
            continue

        # late pools reuse the freed phase-1 space
        wpx_pool = ctx.enter_context(tc.tile_pool(name="wpx", bufs=1))
        sm_pool = ctx.enter_context(tc.tile_pool(name="sm", bufs=4))
        ob_pool = ctx.enter_context(tc.tile_pool(name="ob", bufs=3))
        wp_sb = wpx_pool.tile([P, DT * C], BF16)
        nc.sync.dma_start(out=wp_sb[:], in_=wp[:, :])

        if "O" in mode:
            nc.vector.memset(xT_sb[:], 0.0)

        # ---- phase 2: attention per head pair ---------------------------
        packed = "U" not in mode
        for j in ([] if "O" in mode else range(DT)):   # head pair j: 2j, 2j+1
            for nh in range(2):                        # chunk pair {2nh,2nh+1}
                avs = [
                    av_pool.tile([P, EXPW], F32, tag="av", name=f"av{j}_{hh}")
                    for hh in range(2)
                ]
                pts = {}
                for mt in range(MT):
                    for hh in range(2):
                        roff = hh * D
                        sc = ps_pool.tile([P, EXPW], F32, tag="big", name="sc")
                        for nn in range(2):
                            nc.tensor.matmul(
                                sc[:, nn * NCH:(nn + 1) * NCH],
                                kT_sb[roff:roff + D,
                                      j * M + mt * P: j * M + (mt + 1) * P],
                                qT_sb[roff:roff + D,
                                      j * N + nh * EXPW + nn * NCH:
                                      j * N + nh * EXPW + (nn + 1) * NCH],
                                start=True,
                                stop=True,
                            )
                        if fp8_av:
                            if (mt % 2) == 0:
                                pts[hh] = pt_pool.tile([P, 2, EXPW], vdt,
                                                       tag="pt", name="pt")
                            # exp(s-4) keeps p~ under the fp8e4m3 max (448)
                            # for any realistic score; softmax is shift-
                            # invariant (the ones-column denominator shifts
                            # identically).
                            nc.scalar.activation(
                                pts[hh][:, mt % 2, :], sc[:], AF.Exp,
                                bias=nbias[:])
                        else:
                            pts[hh] = pt_pool.tile([P, 1, EXPW], vdt,
                                                   tag="pt", name="pt")
                            nc.scalar.activation(
                                pts[hh][:, 0, :], sc[:], AF.Exp,
                                bias=nbias[:])

                        h = 2 * j + hh
                        if fp8_av and (mt % 2) == 1:
                            for nn in range(2):
                                nc.tensor.matmul(
                                    avs[hh][:VA, nn * NCH:(nn + 1) * NCH],
                                    va4[:, mt // 2, h, :, :],
                                    pts[hh][:, :, nn * NCH:(nn + 1) * NCH],
                                    start=(mt == 1),
                                    stop=(mt == MT - 1),
                                    perf_mode=mybir.MatmulPerfMode.DoubleRow,
                                )
                        elif not fp8_av:
                            for nn in range(2):
                                nc.tensor.matmul(
                                    avs[hh][:VA, nn * NCH:(nn + 1) * NCH],
                                    va4[:, mt // 2, h, mt % 2, :],
                                    pts[hh][:, 0, nn * NCH:(nn + 1) * NCH],
                                    start=(mt == 0),
                                    stop=(mt == MT - 1),
                                )

                for hh in range(2):
                    h = 2 * j + hh
                    roff = hh * D
                    for cc in range(2):
                        c = nh * 2 + cc
                        rc = sm_pool.tile([1, NCH], F32, tag="rc", name="rc")
                        nc.vector.reciprocal(
                            rc[:], avs[hh][D:VA, cc * NCH:(cc + 1) * NCH])
                        bc = sm_pool.tile([D, NCH], F32, tag="bc", name="bc")
                        nc.gpsimd.partition_broadcast(bc[:], rc[:])
                        nc.vector.tensor_mul(
                            xT_sb[roff:roff + D,
                                  j * N + c * NCH: j * N + (c + 1) * NCH],
                            avs[hh][:D, cc * NCH:(cc + 1) * NCH],
                            bc[:],
                        )

        if "A" in mode:
            continue
        # ---- phase 3: output projection (partial over local channels) ---
        for mt8 in range(CT):
            ob = ob_pool.tile([P, N], F32, tag="ob", name="ob")
            for ch in range(NCHUNKS):
                acc = ps_pool.tile([P, EXPW], F32, tag="big", name="po")
                for ct in range(DT):
                    nc.tensor.matmul(
                        acc[:, :NCH],
                        wp_sb[:, ct * C + mt8 * P: ct * C + (mt8 + 1) * P],
                        xT_sb[:, ct * N + ch * NCH: ct * N + (ch + 1) * NCH],
                        start=(ct == 0),
                        stop=(ct == DT - 1),
                    )
                nc.vector.tensor_scalar_add(
                    ob[:, ch * NCH:(ch + 1) * NCH], acc[:, :NCH],
                    bp_sb[:, mt8:mt8 + 1])
            if "X" not in mode:
                nc.sync.dma_start(out=out[mt8 * P:(mt8 + 1) * P, :], in_=ob[:])

    nc.compile()
    return nc


_NC_CACHE = {}


def _get_program(reps: int = 1, mode: str = ""):
    key = (reps, mode)
    if key not in _NC_CACHE:
        _NC_CACHE[key] = build_program(reps, mode)
    return _NC_CACHE[key]


def _tile_w(wT_slice):
    """(C, CL) weight slice -> pre-tiled (P, CT*CL) SBUF image."""
    c, cl = wT_slice.shape
    return np.ascontiguousarray(
        wT_slice.reshape(c // P, P, cl).transpose(1, 0, 2).reshape(P, -1))


def make_in_maps(query, key, value, Wq, Wk, Wv, Wp, bp):
    query = np.asarray(query, dtype=np.float32)
    key = np.asarray(key, dtype=np.float32)
    value = np.asarray(value, dtype=np.float32)
    Wq = np.asarray(Wq, dtype=np.float32)
    Wk = np.asarray(Wk, dtype=np.float32)
    Wv = np.asarray(Wv, dtype=np.float32)
    Wp = np.asarray(Wp, dtype=np.float32)
    bp = np.asarray(bp, dtype=np.float32)

    wqT = np.ascontiguousarray(Wq.T) * np.float32(SCALE)  # (C, C)
    wkT = np.ascontiguousarray(Wk.T)
    wvT = np.ascontiguousarray(Wv.T)
    wpT = np.ascontiguousarray(Wp.T)                      # (C, C)
    zeros_bp = np.zeros_like(bp)

    in_maps = []
    for core in range(8):
        b, g = divmod(core, 2)
        sl = slice(g * CL, (g + 1) * CL)
        bpc = (bp if g == 0 else zeros_bp)
        in_maps.append({
            "qTin": np.ascontiguousarray(query[b].T),
            "kTin": np.ascontiguousarray(key[b].T),
            "vTin": np.ascontiguousarray(value[b].T),
            "wq": _tile_w(wqT[:, sl]),
            "wk": _tile_w(wkT[:, sl]),
            "wv": _tile_w(wvT[:, sl]),
            "wp": _tile_w(wpT[sl, :]).astype(ml_dtypes.bfloat16),
            "bp": np.ascontiguousarray(bpc.reshape(CT, P).T),
        })
    return in_maps


def combine_outputs(results):
    out = np.empty((B, N, C), dtype=np.float32)
    for b in range(B):
        part = results[2 * b]["out"] + results[2 * b + 1]["out"]  # (C, N)
        out[b] = part.T
    return out


def kernel(**inputs) -> np.ndarray:
    nc = _get_program()
    in_maps = make_in_maps(**inputs)
    res = run_bass_kernel_spmd(nc, in_maps, list(range(8)))
    return combine_outputs(res.results)


if __name__ == "__main__":
    nc = _get_program()
    print("program built ok")



# revision 21
# speedup vs baseline: 896.1852x; 896.1852x over previous
"""CrossAttention (B=4, N=M=2048, C=1024, H=16, D=64) on 8 TRN2 cores.

Sharding: core = 2*b + g  (b = batch 0..3, g = head-half 0..1, 8 heads each).
Each core computes attention for its 8 heads and a partial (full-width)
output projection over its 512 local channels; the host sums the two
partials per batch (fp32) and transposes back.

v2 design (vs the phase-serial v1):
  - All operands bf16 (fp32 PSUM accumulation everywhere): halves DMA
    volume, enables FWL on the weight path, rel-err ~1e-2 < 2e-2 gate.
  - Projections use full-depth 8-tile accumulation chains (no DVE
    copy+add halves), 1024-wide PSUM chunks drained by one DVE copy.
  - Software-pipelined emission: v-projection, q-projection (pairs 1-3)
    and the output projection are woven between the attention blocks'
    score/exp/AV groups, so the PE stays dense while the scalar engine
    (exp, the 2nd-busiest engine) runs the attention cadence.
  - Scores put keys on partitions (m on partitions, n free) so AV needs
    no transposes; the two heads of a pair sit on partitions 0-63 /
    64-127 and run as row-tiled matmuls.
  - Softmax denominator comes free from a ones-column appended to V
    (65-col AV stationary; row 64 of the accumulator is sum_m exp(s)).
  - Normalization: DVE reciprocal + GPSIMD partition-broadcast + DVE
    multiply into xT (bf16).
"""

from contextlib import ExitStack

import ml_dtypes
import numpy as np

import concourse.bass as bass
import concourse.mybir as mybir
import concourse.tile as tile
from concourse import bacc, library_config
from concourse.bass_utils import run_bass_kernel_spmd

dt = mybir.dt
AF = mybir.ActivationFunctionType

# Problem dims (hardcoded; must match the harness inputs).
B, N, M, C, H = 4, 2048, 2048, 1024, 16
D = C // H            # 64
SCALE = D ** -0.5     # 0.125 (exact)
CL = C // 2           # 512 channels per core (8 heads)
HL = H // 2           # 8 local heads
P = 128
CT = C // P           # 8 input-channel tiles
DT = CL // P          # 4 local-channel tiles
MT = M // P           # 16 key tiles
NCH = 512             # psum bank width in fp32
NCHUNKS = N // NCH    # 4
EXPW = 1024           # exp width (2 psum banks)
VA = D + 1            # 65: v columns + ones column

F32 = dt.float32
BF16 = dt.bfloat16


def build_program(reps: int = 1, mode: str = "") -> bass.Bass:
    """reps>1 repeats the whole body for timing (wall-time delta isolates
    device time from host/transfer overhead).

    mode flags (diagnostics): 'P' stop after projections, 'A' stop after
    attention, 'O' skip attention (zero xT), 'X' skip final output DMA,
    'S' no weaving (serial phases)."""
    nc = bacc.Bacc()
    nc.gpsimd.load_library(library_config.attn)

    qTin = nc.declare_dram_parameter("qTin", [C, N], BF16, isOutput=False)
    kTin = nc.declare_dram_parameter("kTin", [C, M], BF16, isOutput=False)
    vTin = nc.declare_dram_parameter("vTin", [C, M], BF16, isOutput=False)
    # weights arrive pre-tiled to SBUF layout: contiguous (P, x) DMAs
    wq = nc.declare_dram_parameter("wq", [P, CT * CL], BF16, isOutput=False)
    wk = nc.declare_dram_parameter("wk", [P, CT * CL], BF16, isOutput=False)
    wv = nc.declare_dram_parameter("wv", [P, CT * CL], BF16, isOutput=False)
    wp = nc.declare_dram_parameter("wp", [P, DT * C], BF16, isOutput=False)
    bp = nc.declare_dram_parameter("bp", [P, CT], F32, isOutput=False)
    out = nc.declare_dram_parameter("out", [C, N], BF16, isOutput=True)

    weave = "S" not in mode

    with tile.TileContext(nc) as tc:
      for _rep in range(reps):
       with ExitStack() as ctx:
        # ---- persistent sbuf tensors -------------------------------------
        const_pool = ctx.enter_context(tc.tile_pool(name="consts", bufs=1))
        bp_sb = const_pool.tile([P, CT], F32)
        nbias = const_pool.tile([P, 1], F32)
        nc.vector.memset(nbias[:], -2.0)
        qT_sb = const_pool.tile([P, DT * N], BF16)   # local q^, transposed
        kT_sb = const_pool.tile([P, DT * M], BF16)   # local k^, transposed
        va_sb = const_pool.tile([P, MT * HL * VA], BF16)  # v^(+ones)
        xT_sb = const_pool.tile([P, DT * N], BF16)   # attention out, transposed

        va3 = va_sb[:].rearrange("p (r h e) -> p r h e", h=HL, e=VA)

        # ---- pools -------------------------------------------------------
        # PSUM (8 banks of 512 f32): sc 2x[P,1024]=4, av 1x[P,1024]=2,
        # weave-projection pool 1x[P,1024]=2.  The attention block runs its
        # two heads as sequential 16-mt passes so a single AV accumulator
        # suffices; woven projection chains get their own pool so they never
        # break the score/exp double-buffering.
        ps_pool = ctx.enter_context(tc.tile_pool(name="ps2", bufs=2, space="PSUM"))
        av_pool = ctx.enter_context(tc.tile_pool(name="av", bufs=1, space="PSUM"))
        pw_pool = ctx.enter_context(tc.tile_pool(name="pw", bufs=1, space="PSUM"))
        in_pool = ctx.enter_context(tc.tile_pool(name="inT", bufs=12))
        vin_pool = ctx.enter_context(tc.tile_pool(name="vin", bufs=8))
        w_pool = ctx.enter_context(tc.tile_pool(name="wcur", bufs=2))
        wv_pool = ctx.enter_context(tc.tile_pool(name="wv", bufs=1))
        wp_pool = ctx.enter_context(tc.tile_pool(name="wpx", bufs=1))
        pt_pool = ctx.enter_context(tc.tile_pool(name="pt", bufs=4))
        sm_pool = ctx.enter_context(tc.tile_pool(name="sm", bufs=4))
        ob_pool = ctx.enter_context(tc.tile_pool(name="ob", bufs=2))

        # ---- queue all input DMAs in consumption order -------------------
        nc.sync.dma_start(out=bp_sb[:], in_=bp[:, :])
        wk_sb = w_pool.tile([P, CT * CL], BF16, tag="w", name="wk")
        nc.sync.dma_start(out=wk_sb[:], in_=wk[:, :])
        ktiles = []
        for ct in range(CT):
            t = in_pool.tile([P, M], BF16, tag="inT", name=f"k{ct}")
            nc.sync.dma_start(out=t[:], in_=kTin[ct * P:(ct + 1) * P, :])
            ktiles.append(t)
        wq_sb = w_pool.tile([P, CT * CL], BF16, tag="w", name="wq")
        nc.sync.dma_start(out=wq_sb[:], in_=wq[:, :])
        # q ct0-3 use the pool's 4 spare bufs (land during k-proj); ct4-7
        # are queued after v/wp so their buffer-wait (on k-tile frees)
        # doesn't delay the v DMAs that block (0,0) needs.
        qtiles = []
        for ct in range(4):
            t = in_pool.tile([P, N], BF16, tag="inT", name=f"q{ct}")
            nc.sync.dma_start(out=t[:], in_=qTin[ct * P:(ct + 1) * P, :])
            qtiles.append(t)
        wv_sb = wv_pool.tile([P, CT * CL], BF16)
        nc.sync.dma_start(out=wv_sb[:], in_=wv[:, :])
        vtiles = []
        for ct in range(CT):
            t = vin_pool.tile([P, M], BF16, tag="vin", name=f"v{ct}")
            nc.sync.dma_start(out=t[:], in_=vTin[ct * P:(ct + 1) * P, :])
            vtiles.append(t)
        wp_sb = wp_pool.tile([P, DT * C], BF16)
        nc.sync.dma_start(out=wp_sb[:], in_=wp[:, :])
        for ct in range(4, CT):
            t = in_pool.tile([P, N], BF16, tag="inT", name=f"q{ct}")
            nc.sync.dma_start(out=t[:], in_=qTin[ct * P:(ct + 1) * P, :])
            qtiles.append(t)

        # ---- PE work items (emitted inline or woven into attention) ------
        # Weave granularity matters: a full 16-MM projection chunk (3.4us)
        # stalls the exp pipeline (only 2 sc bufs of lookahead), so woven
        # work is split into ~0.85us half-chain sub-items.
        def qk_subs(tiles, w_sb, dst_sb, j, c2, pool=None):
            """[P, 1024] projection chunk as 4 sub-items (4 MMs each) plus
            a DVE drain folded into the last."""
            state = {}

            def sub(nn, half):
                if "acc" not in state:
                    state["acc"] = (pool or pw_pool).tile(
                        [P, EXPW], F32, tag="big", name="prj")
                acc = state["acc"]
                for ct in range(half * 4, half * 4 + 4):
                    nc.tensor.matmul(
                        acc[:, nn * NCH:(nn + 1) * NCH],
                        w_sb[:, ct * CL + j * P: ct * CL + (j + 1) * P],
                        tiles[ct][:, c2 * EXPW + nn * NCH:
                                  c2 * EXPW + (nn + 1) * NCH],
                        start=(ct == 0),
                        stop=(ct == CT - 1),
                    )
                if nn == 1 and half == 1:
                    nc.vector.tensor_copy(
                        dst_sb[:, j * N + c2 * EXPW:
                               j * N + (c2 + 1) * EXPW], acc[:])

            return [(lambda nn=nn, half=half: sub(nn, half))
                    for nn in range(2) for half in range(2)]

        def qk_chunk(tiles, w_sb, dst_sb, j, c2, pool=None):
            for f in qk_subs(tiles, w_sb, dst_sb, j, c2, pool):
                f()

        def v_subs(mt, pool=None):
            """v^ for key-tile mt (8 local heads + ones col), 2 sub-items."""
            state = {}

            def sub(half):
                if "acc" not in state:
                    state["acc"] = (pool or pw_pool).tile(
                        [P, EXPW], F32, tag="big", name="prv")
                acc = state["acc"]
                for ct in range(half * 4, half * 4 + 4):
                    nc.tensor.matmul(
                        acc[:, :CL],
                        vtiles[ct][:, mt * P:(mt + 1) * P],
                        wv_sb[:, ct * CL:(ct + 1) * CL],
                        start=(ct == 0),
                        stop=(ct == CT - 1),
                    )
                if half == 1:
                    blk = va3[:, mt, :, :]            # (P, HL, VA)
                    nc.vector.tensor_copy(
                        blk[:, :, :D],
                        acc[:, :CL].rearrange("p (h d) -> p h d", d=D),
                    )
                    nc.vector.memset(blk[:, :, D:VA], 1.0)

            return [(lambda half=half: sub(half)) for half in range(2)]

        def v_mt(mt, pool=None):
            for f in v_subs(mt, pool):
                f()

        def out_chunk(mt8, ch, pool=None, ch2=None, tag="big"):
            """Partial output projection for out-channel tile mt8.  With
            ch: one 512-query chunk; with ch2: a 1024-wide pair of chunks
            (one DVE drain, for the tail where pools are plentiful)."""
            acc = (pool or pw_pool).tile([P, EXPW], F32, tag=tag, name="po")
            chunks = [ch] if ch2 is None else [2 * ch2, 2 * ch2 + 1]
            for i, c in enumerate(chunks):
                for ct in range(DT):
                    nc.tensor.matmul(
                        acc[:, i * NCH:(i + 1) * NCH],
                        wp_sb[:, ct * C + mt8 * P: ct * C + (mt8 + 1) * P],
                        xT_sb[:, ct * N + c * NCH: ct * N + (c + 1) * NCH],
                        start=(ct == 0),
                        stop=(ct == DT - 1),
                    )
            w = len(chunks) * NCH
            ob = ob_pool.tile([P, EXPW], BF16, tag="ob", name="ob")
            nc.vector.tensor_scalar_add(ob[:, :w], acc[:, :w],
                                        bp_sb[:, mt8:mt8 + 1])
            if "X" not in mode:
                c0 = chunks[0]
                nc.sync.dma_start(
                    out=out[mt8 * P:(mt8 + 1) * P,
                            c0 * NCH: c0 * NCH + w],
                    in_=ob[:, :w])

        # ---- attention block: pair j, 512-query chunk qc -----------------
        # One [P,1024] sc tile holds BOTH heads' scores for the chunk
        # (hh0 in cols 0:512, hh1 in 512:1024): a single 1024-wide exp
        # serves both heads, one [P,1024] AV accumulator holds both heads'
        # AV, and the two score matmuls land on disjoint PE row-groups
        # (0-63 / 64-127) back-to-back — concurrent on hardware.
        def block(j, qc, weave_items):
            """weave_items: list of (step, fn), step in 0..15."""
            avs = av_pool.tile([P, EXPW], F32, tag="av", name=f"av{j}_{qc}")
            pts = {}

            def av_mm(mt):
                pt = pts.pop(mt)
                for hh in range(2):
                    nc.tensor.matmul(
                        avs[:VA, hh * NCH:(hh + 1) * NCH],
                        va3[:, mt, 2 * j + hh, :],
                        pt[:, hh * NCH:(hh + 1) * NCH],
                        start=(mt == 0),
                        stop=(mt == MT - 1),
                    )

            wi = 0
            for mt in range(MT):
                while wi < len(weave_items) and weave_items[wi][0] <= mt:
                    weave_items[wi][1]()
                    wi += 1
                sc = ps_pool.tile([P, EXPW], F32, tag="big", name="sc")
                for hh in range(2):
                    roff = hh * D
                    nc.tensor.matmul(
                        sc[:, hh * NCH:(hh + 1) * NCH],
                        kT_sb[roff:roff + D,
                              j * M + mt * P: j * M + (mt + 1) * P],
                        qT_sb[roff:roff + D,
                              j * N + qc * NCH: j * N + (qc + 1) * NCH],
                        start=True,
                        stop=True,
                    )
                pt = pt_pool.tile([P, EXPW], BF16, tag="pt", name="pt")
                nc.scalar.activation(pt[:], sc[:], AF.Exp, bias=nbias[:])
                pts[mt] = pt
                # 2-step lag: AV for mt-2 — its exp finished during the last
                # two score groups, so the PE queue head never waits
                if mt >= 2:
                    av_mm(mt - 2)
            av_mm(MT - 2)
            av_mm(MT - 1)
            while wi < len(weave_items):
                weave_items[wi][1]()
                wi += 1
            for hh in range(2):
                roff = hh * D
                rc = sm_pool.tile([1, NCH], F32, tag="rc", name="rc")
                nc.vector.reciprocal(
                    rc[:], avs[D:VA, hh * NCH:(hh + 1) * NCH])
                bc = sm_pool.tile([D, NCH], F32, tag="bc", name="bc")
                nc.gpsimd.partition_broadcast(bc[:], rc[:])
                nc.vector.tensor_mul(
                    xT_sb[roff:roff + D,
                          j * N + qc * NCH: j * N + (qc + 1) * NCH],
                    avs[:D, hh * NCH:(hh + 1) * NCH],
                    bc[:],
                )

        # ---- emission ----------------------------------------------------
        # k fully (block (0,0) needs all of kT[0], and later pairs follow
        # soon); then q pair 0; the rest is woven.  Pre-attention chunks
        # alternate between the (still idle) sc pool and the weave pool.
        pre = [ps_pool, pw_pool]
        pi = 0
        for j in range(DT):
            for c2 in range(2):
                qk_chunk(ktiles, wk_sb, kT_sb, j, c2, pool=pre[pi % 2]); pi += 1
        qk_chunk(qtiles, wq_sb, qT_sb, 0, 0, pool=pre[pi % 2]); pi += 1

        if "O" in mode:
            nc.vector.memset(xT_sb[:], 0.0)

        if not weave or "P" in mode:
            # serial order: all projections, then attention, then out-proj
            qk_chunk(qtiles, wq_sb, qT_sb, 0, 1, pool=pre[pi % 2]); pi += 1
            for j in range(1, DT):
                for c2 in range(2):
                    qk_chunk(qtiles, wq_sb, qT_sb, j, c2, pool=pre[pi % 2])
                    pi += 1
            for mt in range(MT):
                v_mt(mt, pool=pre[pi % 2]); pi += 1
            if "P" in mode:
                continue
            if "O" not in mode:
                for nh in range(2):
                    for j in range(DT):
                        block(j, nh, [])
            if "A" in mode:
                continue
            for ch in range(NCHUNKS):
                for mt8 in range(CT):
                    out_chunk(mt8, ch)
            continue

        # weave schedule (steps 0..31 per block; hh0 pass = steps 0..15):
        #   block (0,0): v_mt sub-pairs 1:1 through pass hh0 (AV with 2-step
        #                lag consumes va[mt] at step mt+2), q(0,1)+q(1,0)
        #                subs through pass hh1
        #   blocks (1,0)-(3,0): remaining q subs, ~1 per 3-4 steps
        #   blocks (*,1): out-proj chunks 0,1 spread over all four
        #   tail: out-proj chunks 2,3
        mk = lambda f, *a: (lambda: f(*a))
        och = lambda mt8, ch: mk(out_chunk, mt8, ch)

        def spread(subs, lo, hi):
            """Assign steps lo..hi evenly to the given sub-items."""
            n = len(subs)
            return [(lo + (i * (hi - lo + 1)) // n, s)
                    for i, s in enumerate(subs)]

        qsub = lambda j, c2: qk_subs(qtiles, wq_sb, qT_sb, j, c2)
        b00 = []
        for mt in range(MT):
            for s in v_subs(mt):
                b00.append((mt, s))
        b00 += spread(qsub(0, 1) + qsub(1, 0), 16, 31)
        b10 = spread(qsub(1, 1) + qsub(2, 0), 2, 31)
        b20 = spread(qsub(2, 1) + qsub(3, 0), 2, 31)
        b30 = spread(qsub(3, 1), 4, 31)
        oc0 = [och(mt8, 0) for mt8 in range(CT)]
        oc1 = [och(mt8, 1) for mt8 in range(CT)]
        b01 = spread(oc0[:4], 2, 31)
        b11 = spread(oc0[4:] + oc1[:2], 2, 31)
        b21 = spread(oc1[2:6], 2, 31)
        b31 = spread(oc1[6:], 4, 24)
        weaves = {(0, 0): b00, (1, 0): b10, (2, 0): b20, (3, 0): b30,
                  (0, 1): b01, (1, 1): b11, (2, 1): b21, (3, 1): b31}
        for nh in range(2):
            for j in range(DT):
                block(j, nh, weaves.get((j, nh), []))
        if "A" in mode:
            continue
        # tail: sc and av pools are idle now — 1024-wide pairs rotating
        # through three pools so no chain ever waits on a drain
        tailpools = [(ps_pool, "big"), (pw_pool, "big"), (av_pool, "av")]
        for i, mt8 in enumerate(range(CT)):
            pool, tag = tailpools[i % 3]
            out_chunk(mt8, None, pool=pool, ch2=1, tag=tag)

    nc.compile()
    return nc


_NC_CACHE = {}


def _get_program(reps: int = 1, mode: str = ""):
    key = (reps, mode)
    if key not in _NC_CACHE:
        _NC_CACHE[key] = build_program(reps, mode)
    return _NC_CACHE[key]


def _tile_w(wT_slice):
    """(C, CL) weight slice -> pre-tiled (P, CT*CL) SBUF image."""
    c, cl = wT_slice.shape
    return np.ascontiguousarray(
        wT_slice.reshape(c // P, P, cl).transpose(1, 0, 2).reshape(P, -1))


def make_in_maps(query, key, value, Wq, Wk, Wv, Wp, bp):
    query = np.asarray(query, dtype=np.float32)
    key = np.asarray(key, dtype=np.float32)
    value = np.asarray(value, dtype=np.float32)
    Wq = np.asarray(Wq, dtype=np.float32)
    Wk = np.asarray(Wk, dtype=np.float32)
    Wv = np.asarray(Wv, dtype=np.float32)
    Wp = np.asarray(Wp, dtype=np.float32)
    bp = np.asarray(bp, dtype=np.float32)
    bf = ml_dtypes.bfloat16

    wqT = np.ascontiguousarray(Wq.T) * np.float32(SCALE)  # (C, C)
    wkT = np.ascontiguousarray(Wk.T)
    wvT = np.ascontiguousarray(Wv.T)
    wpT = np.ascontiguousarray(Wp.T)                      # (C, C)
    zeros_bp = np.zeros_like(bp)

    qT = [np.ascontiguousarray(query[b].T).astype(bf) for b in range(B)]
    kT = [np.ascontiguousarray(key[b].T).astype(bf) for b in range(B)]
    vT = [np.ascontiguousarray(value[b].T).astype(bf) for b in range(B)]

    in_maps = []
    for core in range(8):
        b, g = divmod(core, 2)
        sl = slice(g * CL, (g + 1) * CL)
        bpc = (bp if g == 0 else zeros_bp)
        in_maps.append({
            "qTin": qT[b],
            "kTin": kT[b],
            "vTin": vT[b],
            "wq": _tile_w(wqT[:, sl]).astype(bf),
            "wk": _tile_w(wkT[:, sl]).astype(bf),
            "wv": _tile_w(wvT[:, sl]).astype(bf),
            "wp": _tile_w(wpT[sl, :]).astype(bf),
            "bp": np.ascontiguousarray(bpc.reshape(CT, P).T),
        })
    return in_maps


def combine_outputs(results):
    out = np.empty((B, N, C), dtype=np.float32)
    for b in range(B):
        part = (results[2 * b]["out"].astype(np.float32)
                + results[2 * b + 1]["out"].astype(np.float32))  # (C, N)
        out[b] = part.T
    return out


def kernel(**inputs) -> np.ndarray:
    nc = _get_program()
    in_maps = make_in_maps(**inputs)
    res = run_bass_kernel_spmd(nc, in_maps, list(range(8)))
    return combine_outputs(res.results)


if __name__ == "__main__":
    nc = _get_program()
    print("program built ok")


# revision 29
# speedup vs baseline: 989.9086x; 1.1046x over previous
"""CrossAttention (B=4, N=M=2048, C=1024, H=16, D=64) on 8 TRN2 cores.

Sharding: core = 2*b + g  (b = batch 0..3, g = head-half 0..1, 8 heads each).
Each core computes attention for its 8 heads and a partial (full-width)
output projection over its 512 local channels; the host sums the two
partials per batch (fp32) and transposes back.

v2 design (vs the phase-serial v1):
  - All operands bf16 (fp32 PSUM accumulation everywhere): halves DMA
    volume, enables FWL on the weight path, rel-err ~1e-2 < 2e-2 gate.
  - Projections use full-depth 8-tile accumulation chains (no DVE
    copy+add halves), 1024-wide PSUM chunks drained by one DVE copy.
  - Software-pipelined emission: v-projection, q-projection (pairs 1-3)
    and the output projection are woven between the attention blocks'
    score/exp/AV groups, so the PE stays dense while the scalar engine
    (exp, the 2nd-busiest engine) runs the attention cadence.
  - Scores put keys on partitions (m on partitions, n free) so AV needs
    no transposes; the two heads of a pair sit on partitions 0-63 /
    64-127 and run as row-tiled matmuls.
  - Softmax denominator comes free from a ones-column appended to V
    (65-col AV stationary; row 64 of the accumulator is sum_m exp(s)).
  - Normalization: DVE reciprocal + GPSIMD partition-broadcast + DVE
    multiply into xT (bf16).
"""

from contextlib import ExitStack

import ml_dtypes
import numpy as np

import concourse.bass as bass
import concourse.mybir as mybir
import concourse.tile as tile
from concourse import bacc, library_config
from concourse.bass_utils import run_bass_kernel_spmd

dt = mybir.dt
AF = mybir.ActivationFunctionType

# Problem dims (hardcoded; must match the harness inputs).
B, N, M, C, H = 4, 2048, 2048, 1024, 16
D = C // H            # 64
SCALE = D ** -0.5     # 0.125 (exact)
CL = C // 2           # 512 channels per core (8 heads)
HL = H // 2           # 8 local heads
P = 128
CT = C // P           # 8 input-channel tiles
DT = CL // P          # 4 local-channel tiles
MT = M // P           # 16 key tiles
NCH = 512             # psum bank width in fp32
NCHUNKS = N // NCH    # 4
EXPW = 1024           # exp width (2 psum banks)
VA = D + 1            # 65: v columns + ones column

F32 = dt.float32
BF16 = dt.bfloat16


def build_program(reps: int = 1, mode: str = "") -> bass.Bass:
    """reps>1 repeats the whole body for timing (wall-time delta isolates
    device time from host/transfer overhead).

    mode flags (diagnostics): 'P' stop after projections, 'A' stop after
    attention, 'O' skip attention (zero xT), 'X' skip final output DMA,
    'S' no weaving (serial phases)."""
    nc = bacc.Bacc()
    nc.gpsimd.load_library(library_config.attn)

    qTin = nc.declare_dram_parameter("qTin", [C, N], BF16, isOutput=False)
    kTin = nc.declare_dram_parameter("kTin", [C, M], BF16, isOutput=False)
    vTin = nc.declare_dram_parameter("vTin", [C, M], BF16, isOutput=False)
    # weights arrive pre-tiled to SBUF layout: contiguous (P, x) DMAs
    wq = nc.declare_dram_parameter("wq", [P, CT * CL], BF16, isOutput=False)
    wk = nc.declare_dram_parameter("wk", [P, CT * CL], BF16, isOutput=False)
    wv = nc.declare_dram_parameter("wv", [P, CT * CL], BF16, isOutput=False)
    wp = nc.declare_dram_parameter("wp", [P, DT * C], BF16, isOutput=False)
    bp = nc.declare_dram_parameter("bp", [P, CT], F32, isOutput=False)
    out = nc.declare_dram_parameter("out", [C, N], BF16, isOutput=True)

    weave = "S" not in mode

    with tile.TileContext(nc) as tc:
      for _rep in range(reps):
       with ExitStack() as ctx:
        # ---- persistent sbuf tensors -------------------------------------
        const_pool = ctx.enter_context(tc.tile_pool(name="consts", bufs=1))
        bp_sb = const_pool.tile([P, CT], F32)
        nbias = const_pool.tile([P, 1], F32)
        nc.vector.memset(nbias[:], -2.0)
        qT_sb = const_pool.tile([P, DT * N], BF16)   # local q^, transposed
        kT_sb = const_pool.tile([P, DT * M], BF16)   # local k^, transposed
        va_sb = const_pool.tile([P, MT * HL * VA], BF16)  # v^(+ones)
        xT_sb = const_pool.tile([P, DT * N], BF16)   # attention out, transposed

        va3 = va_sb[:].rearrange("p (r h e) -> p r h e", h=HL, e=VA)

        # ---- pools -------------------------------------------------------
        # PSUM (8 banks of 512 f32): sc 2x[P,1024]=4, av 1x[P,1024]=2,
        # weave-projection pool 1x[P,1024]=2.  The attention block runs its
        # two heads as sequential 16-mt passes so a single AV accumulator
        # suffices; woven projection chains get their own pool so they never
        # break the score/exp double-buffering.
        ps_pool = ctx.enter_context(tc.tile_pool(name="ps2", bufs=2, space="PSUM"))
        av_pool = ctx.enter_context(tc.tile_pool(name="av", bufs=1, space="PSUM"))
        pw_pool = ctx.enter_context(tc.tile_pool(name="pw", bufs=1, space="PSUM"))
        in_pool = ctx.enter_context(tc.tile_pool(name="inT", bufs=12))
        vin_pool = ctx.enter_context(tc.tile_pool(name="vin", bufs=8))
        w_pool = ctx.enter_context(tc.tile_pool(name="wcur", bufs=2))
        wv_pool = ctx.enter_context(tc.tile_pool(name="wv", bufs=1))
        wp_pool = ctx.enter_context(tc.tile_pool(name="wpx", bufs=1))
        pt_pool = ctx.enter_context(tc.tile_pool(name="pt", bufs=4))
        sm_pool = ctx.enter_context(tc.tile_pool(name="sm", bufs=2))
        sn_pool = ctx.enter_context(tc.tile_pool(name="sn", bufs=2))
        ob_pool = ctx.enter_context(tc.tile_pool(name="ob", bufs=2))

        # ---- queue all input DMAs in consumption order -------------------
        nc.sync.dma_start(out=bp_sb[:], in_=bp[:, :])
        wk_sb = w_pool.tile([P, CT * CL], BF16, tag="w", name="wk")
        # wk in halves so the first chain's ct0-3 matmuls start sooner
        nc.sync.dma_start(out=wk_sb[:, :4 * CL], in_=wk[:, :4 * CL])
        ktiles = []
        for ct in range(CT):
            t = in_pool.tile([P, M], BF16, tag="inT", name=f"k{ct}")
            nc.sync.dma_start(out=t[:], in_=kTin[ct * P:(ct + 1) * P, :])
            ktiles.append(t)
            if ct == 1:
                nc.sync.dma_start(out=wk_sb[:, 4 * CL:], in_=wk[:, 4 * CL:])
        # v before q: block (0,0)'s woven v-projection consumes v tiles
        # right at attention start, while q chunks for later blocks can lag.
        wv_sb = wv_pool.tile([P, CT * CL], BF16)
        nc.sync.dma_start(out=wv_sb[:], in_=wv[:, :])
        vtiles = []
        for ct in range(CT):
            t = vin_pool.tile([P, M], BF16, tag="vin", name=f"v{ct}")
            nc.sync.dma_start(out=t[:], in_=vTin[ct * P:(ct + 1) * P, :])
            vtiles.append(t)
        wq_sb = w_pool.tile([P, CT * CL], BF16, tag="w", name="wq")
        nc.sync.dma_start(out=wq_sb[:], in_=wq[:, :])
        # q ct0-3 use the pool's 4 spare bufs; ct4-7 rotate onto k-tile
        # buffers (freed when k-proj's last chain reads them)
        qtiles = []
        for ct in range(CT):
            t = in_pool.tile([P, N], BF16, tag="inT", name=f"q{ct}")
            nc.sync.dma_start(out=t[:], in_=qTin[ct * P:(ct + 1) * P, :])
            qtiles.append(t)
        wp_sb = wp_pool.tile([P, DT * C], BF16)
        nc.sync.dma_start(out=wp_sb[:], in_=wp[:, :])

        # ---- PE work items (emitted inline or woven into attention) ------
        # Weave granularity matters: a full 16-MM projection chunk (3.4us)
        # stalls the exp pipeline (only 2 sc bufs of lookahead), so woven
        # work is split into ~0.85us half-chain sub-items.
        def qk_subs(tiles, w_sb, dst_sb, j, c2, pool=None):
            """[P, 1024] projection chunk as 4 sub-items (4 MMs each) plus
            a DVE drain folded into the last."""
            state = {}

            def sub(nn, half):
                if "acc" not in state:
                    state["acc"] = (pool or pw_pool).tile(
                        [P, EXPW], F32, tag="big", name="prj")
                acc = state["acc"]
                for ct in range(half * 4, half * 4 + 4):
                    nc.tensor.matmul(
                        acc[:, nn * NCH:(nn + 1) * NCH],
                        w_sb[:, ct * CL + j * P: ct * CL + (j + 1) * P],
                        tiles[ct][:, c2 * EXPW + nn * NCH:
                                  c2 * EXPW + (nn + 1) * NCH],
                        start=(ct == 0),
                        stop=(ct == CT - 1),
                    )
                if nn == 1 and half == 1:
                    nc.vector.tensor_copy(
                        dst_sb[:, j * N + c2 * EXPW:
                               j * N + (c2 + 1) * EXPW], acc[:])

            return [(lambda nn=nn, half=half: sub(nn, half))
                    for nn in range(2) for half in range(2)]

        def qk_chunk(tiles, w_sb, dst_sb, j, c2, pool=None):
            for f in qk_subs(tiles, w_sb, dst_sb, j, c2, pool):
                f()

        def v_subs(mt, pool=None):
            """v^ for key-tile mt (8 local heads + ones col), 2 sub-items."""
            state = {}

            def sub(half):
                if "acc" not in state:
                    state["acc"] = (pool or pw_pool).tile(
                        [P, EXPW], F32, tag="big", name="prv")
                acc = state["acc"]
                for ct in range(half * 4, half * 4 + 4):
                    nc.tensor.matmul(
                        acc[:, :CL],
                        vtiles[ct][:, mt * P:(mt + 1) * P],
                        wv_sb[:, ct * CL:(ct + 1) * CL],
                        start=(ct == 0),
                        stop=(ct == CT - 1),
                    )
                if half == 1:
                    blk = va3[:, mt, :, :]            # (P, HL, VA)
                    nc.vector.tensor_copy(
                        blk[:, :, :D],
                        acc[:, :CL].rearrange("p (h d) -> p h d", d=D),
                    )
                    nc.vector.memset(blk[:, :, D:VA], 1.0)

            return [(lambda half=half: sub(half)) for half in range(2)]

        def v_mt(mt, pool=None):
            for f in v_subs(mt, pool):
                f()

        def out_chunk(mt8, ch, pool=None, ch2=None, tag="big"):
            """Partial output projection for out-channel tile mt8.  With
            ch: one 512-query chunk; with ch2: a 1024-wide pair of chunks
            (one DVE drain, for the tail where pools are plentiful)."""
            acc = (pool or pw_pool).tile([P, EXPW], F32, tag=tag, name="po")
            chunks = [ch] if ch2 is None else [2 * ch2, 2 * ch2 + 1]
            for i, c in enumerate(chunks):
                for ct in range(DT):
                    nc.tensor.matmul(
                        acc[:, i * NCH:(i + 1) * NCH],
                        wp_sb[:, ct * C + mt8 * P: ct * C + (mt8 + 1) * P],
                        xT_sb[:, ct * N + c * NCH: ct * N + (c + 1) * NCH],
                        start=(ct == 0),
                        stop=(ct == DT - 1),
                    )
            w = len(chunks) * NCH
            ob = ob_pool.tile([P, EXPW], BF16, tag="ob", name="ob")
            nc.vector.tensor_scalar_add(ob[:, :w], acc[:, :w],
                                        bp_sb[:, mt8:mt8 + 1])
            if "X" not in mode:
                c0 = chunks[0]
                nc.sync.dma_start(
                    out=out[mt8 * P:(mt8 + 1) * P,
                            c0 * NCH: c0 * NCH + w],
                    in_=ob[:, :w])

        # ---- attention block: pair j, 512-query chunk qc -----------------
        # One [P,1024] sc tile holds BOTH heads' scores for the chunk
        # (hh0 in cols 0:512, hh1 in 512:1024): a single 1024-wide exp
        # serves both heads, one [P,1024] AV accumulator holds both heads'
        # AV, and the two score matmuls land on disjoint PE row-groups
        # (0-63 / 64-127) back-to-back — concurrent on hardware.
        def block(j, qc, weave_items):
            """weave_items: list of (step, fn), step in 0..15."""
            avs = av_pool.tile([P, EXPW], F32, tag="av", name=f"av{j}_{qc}")
            pts = {}

            def av_mm(mt):
                pt = pts.pop(mt)
                for hh in range(2):
                    nc.tensor.matmul(
                        avs[:VA, hh * NCH:(hh + 1) * NCH],
                        va3[:, mt, 2 * j + hh, :],
                        pt[:, hh * NCH:(hh + 1) * NCH],
                        start=(mt == 0),
                        stop=(mt == MT - 1),
                    )

            wi = 0
            for mt in range(MT):
                while wi < len(weave_items) and weave_items[wi][0] <= mt:
                    weave_items[wi][1]()
                    wi += 1
                sc = ps_pool.tile([P, EXPW], F32, tag="big", name="sc")
                for hh in range(2):
                    roff = hh * D
                    nc.tensor.matmul(
                        sc[:, hh * NCH:(hh + 1) * NCH],
                        kT_sb[roff:roff + D,
                              j * M + mt * P: j * M + (mt + 1) * P],
                        qT_sb[roff:roff + D,
                              j * N + qc * NCH: j * N + (qc + 1) * NCH],
                        start=True,
                        stop=True,
                    )
                pt = pt_pool.tile([P, EXPW], BF16, tag="pt", name="pt")
                nc.scalar.activation(pt[:], sc[:], AF.Exp, bias=nbias[:])
                pts[mt] = pt
                # 2-step lag: AV for mt-2 — its exp finished during the last
                # two score groups, so the PE queue head never waits
                if mt >= 2:
                    av_mm(mt - 2)
            av_mm(MT - 2)
            av_mm(MT - 1)
            while wi < len(weave_items):
                weave_items[wi][1]()
                wi += 1
            # one DVE copy frees the AV psum banks ~2us earlier than the
            # recip/broadcast/mul chain would; normalize runs off SBUF
            snap = sn_pool.tile([VA, EXPW], F32, tag="sn", name="snap")
            nc.vector.tensor_copy(snap[:], avs[:VA, :])
            for hh in range(2):
                roff = hh * D
                rc = sm_pool.tile([1, NCH], F32, tag="rc", name="rc")
                nc.vector.reciprocal(
                    rc[:], snap[D:VA, hh * NCH:(hh + 1) * NCH])
                bc = sm_pool.tile([D, NCH], F32, tag="bc", name="bc")
                nc.gpsimd.partition_broadcast(bc[:], rc[:])
                nc.vector.tensor_mul(
                    xT_sb[roff:roff + D,
                          j * N + qc * NCH: j * N + (qc + 1) * NCH],
                    snap[:D, hh * NCH:(hh + 1) * NCH],
                    bc[:],
                )

        # ---- emission ----------------------------------------------------
        # k fully (block (0,0) needs all of kT[0], and later pairs follow
        # soon); then q pair 0; the rest is woven.  Pre-attention chunks
        # alternate between the (still idle) sc pool and the weave pool.
        pre = [ps_pool, pw_pool]
        pi = 0
        for j in range(DT):
            for c2 in range(2):
                qk_chunk(ktiles, wk_sb, kT_sb, j, c2, pool=pre[pi % 2]); pi += 1
        qk_chunk(qtiles, wq_sb, qT_sb, 0, 0, pool=pre[pi % 2]); pi += 1
        if weave:
            qk_chunk(qtiles, wq_sb, qT_sb, 1, 0, pool=pre[pi % 2]); pi += 1

        if "O" in mode:
            nc.vector.memset(xT_sb[:], 0.0)

        if not weave or "P" in mode:
            # serial order: all projections, then attention, then out-proj
            qk_chunk(qtiles, wq_sb, qT_sb, 0, 1, pool=pre[pi % 2]); pi += 1
            for j in range(1, DT):
                for c2 in range(2):
                    qk_chunk(qtiles, wq_sb, qT_sb, j, c2, pool=pre[pi % 2])
                    pi += 1
            for mt in range(MT):
                v_mt(mt, pool=pre[pi % 2]); pi += 1
            if "P" in mode:
                continue
            if "O" not in mode:
                for qc in range(NCHUNKS):
                    for j in range(DT):
                        block(j, qc, [])
            if "A" in mode:
                continue
            for ch in range(NCHUNKS):
                for mt8 in range(CT):
                    out_chunk(mt8, ch)
            continue

        # weave schedule: 16 blocks (qc-major), 16 steps each.
        #   block (0,0): v_mt sub-pairs 1:1 (AV's 2-step lag consumes va[mt]
        #                at step mt+2)
        #   blocks 1-3:  remaining qc=0/1 q-projections
        #   blocks 4-7 (qc=1): q c2=1 chunks + out ch0
        #   blocks 8-11 (qc=2): out ch1 (+ last q)
        #   blocks 12-15 (qc=3): out ch2;  tail: out ch3
        mk = lambda f, *a: (lambda: f(*a))
        och = lambda mt8, ch: mk(out_chunk, mt8, ch)

        def spread(subs, lo, hi):
            """Assign steps lo..hi evenly to the given sub-items."""
            n = len(subs)
            return [(lo + (i * (hi - lo + 1)) // n, s)
                    for i, s in enumerate(subs)]

        qsub = lambda j, c2: qk_subs(qtiles, wq_sb, qT_sb, j, c2)
        oc = {ch: [och(mt8, ch) for mt8 in range(CT)] for ch in range(4)}
        weaves = {}
        # each q chunk lands at least one block before its first consumer
        weaves[(0, 0)] = [(mt, s) for mt in range(MT) for s in v_subs(mt)]
        weaves[(1, 0)] = spread(qsub(2, 0), 2, 15)
        weaves[(2, 0)] = spread(qsub(3, 0), 2, 15)
        weaves[(3, 0)] = spread(qsub(0, 1), 2, 15)
        weaves[(0, 1)] = spread(qsub(1, 1) + [oc[0][0]], 2, 15)
        weaves[(1, 1)] = spread(qsub(2, 1) + [oc[0][1]], 2, 15)
        weaves[(2, 1)] = spread(qsub(3, 1) + [oc[0][2]], 2, 15)
        weaves[(3, 1)] = spread(oc[0][3:6], 2, 15)
        weaves[(0, 2)] = spread(oc[0][6:8] + [oc[1][0]], 3, 15)
        weaves[(1, 2)] = spread(oc[1][1:4], 3, 15)
        weaves[(2, 2)] = spread(oc[1][4:6], 3, 15)
        weaves[(3, 2)] = spread(oc[1][6:8], 3, 15)
        for i, j in enumerate(range(DT)):            # qc=3 blocks
            weaves[(j, 3)] = spread(oc[2][2 * i:2 * i + 2], 3, 15)
        for qc in range(NCHUNKS):
            for j in range(DT):
                block(j, qc, weaves.get((j, qc), []))
        if "A" in mode:
            continue
        # tail: out ch3 — rotate three idle pools so no chain waits a drain
        tailpools = [(ps_pool, "big"), (pw_pool, "big"), (av_pool, "av")]
        for i, mt8 in enumerate(range(CT)):
            pool, tag = tailpools[i % 3]
            out_chunk(mt8, 3, pool=pool, tag=tag)

    nc.compile()
    return nc


_NC_CACHE = {}


def _get_program(reps: int = 1, mode: str = ""):
    key = (reps, mode)
    if key not in _NC_CACHE:
        _NC_CACHE[key] = build_program(reps, mode)
    return _NC_CACHE[key]


def _tile_w(wT_slice):
    """(C, CL) weight slice -> pre-tiled (P, CT*CL) SBUF image."""
    c, cl = wT_slice.shape
    return np.ascontiguousarray(
        wT_slice.reshape(c // P, P, cl).transpose(1, 0, 2).reshape(P, -1))


def make_in_maps(query, key, value, Wq, Wk, Wv, Wp, bp):
    query = np.asarray(query, dtype=np.float32)
    key = np.asarray(key, dtype=np.float32)
    value = np.asarray(value, dtype=np.float32)
    Wq = np.asarray(Wq, dtype=np.float32)
    Wk = np.asarray(Wk, dtype=np.float32)
    Wv = np.asarray(Wv, dtype=np.float32)
    Wp = np.asarray(Wp, dtype=np.float32)
    bp = np.asarray(bp, dtype=np.float32)
    bf = ml_dtypes.bfloat16

    wqT = np.ascontiguousarray(Wq.T) * np.float32(SCALE)  # (C, C)
    wkT = np.ascontiguousarray(Wk.T)
    wvT = np.ascontiguousarray(Wv.T)
    wpT = np.ascontiguousarray(Wp.T)                      # (C, C)
    zeros_bp = np.zeros_like(bp)

    qT = [np.ascontiguousarray(query[b].T).astype(bf) for b in range(B)]
    kT = [np.ascontiguousarray(key[b].T).astype(bf) for b in range(B)]
    vT = [np.ascontiguousarray(value[b].T).astype(bf) for b in range(B)]

    in_maps = []
    for core in range(8):
        b, g = divmod(core, 2)
        sl = slice(g * CL, (g + 1) * CL)
        bpc = (bp if g == 0 else zeros_bp)
        in_maps.append({
            "qTin": qT[b],
            "kTin": kT[b],
            "vTin": vT[b],
            "wq": _tile_w(wqT[:, sl]).astype(bf),
            "wk": _tile_w(wkT[:, sl]).astype(bf),
            "wv": _tile_w(wvT[:, sl]).astype(bf),
            "wp": _tile_w(wpT[sl, :]).astype(bf),
            "bp": np.ascontiguousarray(bpc.reshape(CT, P).T),
        })
    return in_maps


def combine_outputs(results):
    out = np.empty((B, N, C), dtype=np.float32)
    for b in range(B):
        part = (results[2 * b]["out"].astype(np.float32)
                + results[2 * b + 1]["out"].astype(np.float32))  # (C, N)
        out[b] = part.T
    return out


def kernel(**inputs) -> np.ndarray:
    nc = _get_program()
    in_maps = make_in_maps(**inputs)
    res = run_bass_kernel_spmd(nc, in_maps, list(range(8)))
    return combine_outputs(res.results)


if __name__ == "__main__":
    nc = _get_program()
    print("program built ok")
